# revision 1
# baseline (speedup 1.0000x reference)
"""FNO block (nn_FNOBlock_48962627175213) as a Bass/Tile kernel on 8 trn2 cores.

Math: only 64 complex rfft modes (32 low + 32 high) survive into out_ft, so
rfft/irfft collapse into skinny DFT matmuls against precomputed fp32 bases.
Data-parallel over batch: each core takes 4 of the 32 batches.

Per-core pipeline (rows = (b, c) b-major, 256 rows of length L=8192):
  1. head: phi = emb @ A^T (all four parts), FiLM MLP, per-batch scaled
     time weights (1+gamma folded into lin_w^T), folded bias vector.
  2. PE-transpose x tiles -> x^T chunks; fwd DFT: RT[modecol, row] +=
     F_chunk^T @ xT_chunk (64 accumulating matmuls).
  3. XS: per (branch, b) transpose-matmuls of RT blocks against runtime
     diagonal matrices built from phi -> folds the complex phi multiply
     into the layout shuffle (out_pos * phi == (x_ft * phi) @ w).
  4. spectral: 128 small matmuls [K=128 (re i, im i), M=64 o, N=4 b].
  5. R2 transposes -> R2f [(d,br,m), (b,o)] = inverse-DFT lhsT.
  6. inverse DFT + time branch accumulate into one PSUM tile; ACT applies
     silu(psum + folded_bias) and output DMAs stream out.
"""
import sys

if '/opt/trn_rl_repo' not in sys.path:
    sys.path.insert(0, '/opt/trn_rl_repo')

import numpy as np

import concourse.bass as bass
import concourse.mybir as mybir
from concourse.tile import TileContext
from concourse.bass_utils import run_bass_kernel_spmd

FP = mybir.dt.float32
BF = mybir.dt.float16  # 2-byte path: fp16 for 8x the mantissa of bf16
AF = mybir.ActivationFunctionType

B, C, L, M, EMB, HID = 32, 64, 8192, 32, 256, 64
K = L // 2 + 1
NEG0 = K - M          # 4065
N_CORES = 8
B_LOC = B // N_CORES  # 4
ROWS = B_LOC * C      # 256


# --------------------------------------------------------------------------
# host-side constant builders
# --------------------------------------------------------------------------
def _build_constants(weights_pos, weights_neg, A_real_pos, A_imag_pos,
                     A_real_neg, A_imag_neg, tm_w1, tm_b1, tm_w2, tm_b2,
                     lin_w, lin_b):
    n = np.arange(L, dtype=np.float64)
    s = 1.0 / np.sqrt(L)

    # fwd DFT basis [8192, 128], col = br*64 + d*32 + m
    F = np.zeros((L, 128), np.float64)
    for br in range(2):
        for m in range(M):
            k = m if br == 0 else NEG0 + m
            ang = 2.0 * np.pi * k * n / L
            F[:, br * 64 + m] = np.cos(ang) * s
            F[:, br * 64 + 32 + m] = -np.sin(ang) * s
    F_sb = F.reshape(64, 128, 128).transpose(1, 0, 2).reshape(128, 64 * 128)
    F_sb = np.ascontiguousarray(F_sb.astype(np.float32), np.float16)

    # inverse basis [128, 8192], row = d*64 + br*32 + m (pocketfft irfft
    # semantics: Im parts of DC and Nyquist are discarded)
    G = np.zeros((128, L), np.float64)
    for br in range(2):
        for m in range(M):
            k = m if br == 0 else NEG0 + m
            ang = 2.0 * np.pi * k * n / L
            if k == 0:
                G[br * 32 + m] = s
            elif k == L // 2:
                G[br * 32 + m] = np.cos(np.pi * n) * s
            else:
                G[br * 32 + m] = 2.0 * np.cos(ang) * s
                G[64 + br * 32 + m] = -2.0 * np.sin(ang) * s
    G = np.ascontiguousarray(G.astype(np.float32), np.float16)

    # spectral weights [128, 8192]: col = ((br*32+m)*2+dout)*64 + o,
    # rows = (din, i); dout=0 -> [wr; -wi], dout=1 -> [wi; wr]
    Wspec = np.zeros((128, 8192), np.float32)
    for br, wfull in ((0, weights_pos), (1, weights_neg)):
        for m in range(M):
            wr = wfull[:, :, m, 0]
            wi = wfull[:, :, m, 1]
            c0 = (br * 32 + m) * 128          # dout=0 block
            c1 = (br * 32 + m) * 128 + 64     # dout=1 block
            Wspec[0:64, c0:c0 + 64] = wr
            Wspec[64:128, c0:c0 + 64] = -wi
            Wspec[0:64, c1:c1 + 64] = wi
            Wspec[64:128, c1:c1 + 64] = wr

    # phi projector [256, 128]: col = d*64 + br*32 + m; then chunk-major
    # repack to [128, 2*128] since SBUF tiles cap at 128 partitions
    Astack = np.zeros((EMB, 128), np.float32)
    Astack[:, 0:32] = A_real_pos.T
    Astack[:, 32:64] = A_real_neg.T
    Astack[:, 64:96] = A_imag_pos.T
    Astack[:, 96:128] = A_imag_neg.T
    Astack = np.ascontiguousarray(
        Astack.reshape(2, 128, 128).transpose(1, 0, 2).reshape(128, 256))

    w1T = tm_w1.T.astype(np.float32)  # [256, 64] -> [128, 2*64]
    w1T = np.ascontiguousarray(
        w1T.reshape(2, 128, 64).transpose(1, 0, 2).reshape(128, 128))

    Wspec = Wspec.astype(np.float16)
    return dict(
        F=F_sb, G=G, W=Wspec, A=Astack,
        w1T=w1T,
        b1=np.ascontiguousarray(tm_b1[:, None], np.float32),
        w2T=np.ascontiguousarray(tm_w2.T, np.float32),
        b2r=np.ascontiguousarray(np.tile(tm_b2, (4, 1)), np.float32),
        lbr=np.ascontiguousarray(np.tile(lin_b, (4, 1)), np.float32),
        lwT2=np.ascontiguousarray(np.tile(lin_w.T, (2, 1)), np.float32),
        ones=np.ones((1, 64), np.float32),
        id128=np.eye(128, dtype=np.float32),
        id128h=np.eye(128, dtype=np.float16),
        idstack=np.ascontiguousarray(np.tile(np.eye(32), (4, 1)), np.float32),
        nidstack=np.ascontiguousarray(np.tile(-np.eye(32), (4, 1)), np.float32),
    )


# --------------------------------------------------------------------------
# walrus workaround: this container's walrus rejects >1 sync-wait on
# TPB_CTRL lowering (Drain/NoOp). Split extra waits onto preceding NOPs.
# --------------------------------------------------------------------------
def _split_multiwait(nc, max_waits=1):
    for f in nc.m.functions:
        for blk in f.blocks:
            new = []
            changed = False
            for inst in blk.instructions:
                si = inst.sync_info
                if (si is not None and len(si.on_wait) > max_waits):
                    waits = list(si.on_wait)
                    head, tail = waits[:-max_waits], waits[-max_waits:]
                    for j, w in enumerate(head):
                        nop = mybir.InstNoOp(name=f"{inst.name}-ws{j}",
                                             ins=[], outs=[])
                        nop.engine = inst.engine
                        nop.sync_info = mybir.SyncInfo(on_wait=[w], on_update=[])
                        new.append(nop)
                    inst.sync_info = mybir.SyncInfo(on_wait=tail,
                                                    on_update=list(si.on_update))
                    changed = True
                new.append(inst)
            if changed:
                blk.instructions = new


# --------------------------------------------------------------------------
# the bass program (input-value independent; built once)
# --------------------------------------------------------------------------
def _build_nc(split=True, sim_safe=False, nrep=1):
    nc = bass.Bass("TRN2")
    d = {}
    for name, shape, dt_ in (
        ("x4", [ROWS, L], BF), ("embT", [128, 2 * B_LOC], FP),
        ("F", [128, 8192], BF), ("G", [128, 8192], BF),
        ("W", [128, 8192], BF), ("A", [128, 256], FP),
        ("w1T", [128, 128], FP), ("b1", [64, 1], FP), ("w2T", [64, 128], FP),
        ("b2r", [4, 128], FP), ("lbr", [4, 64], FP), ("lwT2", [128, 64], FP),
        ("ones", [1, 64], FP), ("id128", [128, 128], FP),
        ("id128h", [128, 128], BF),
        ("idstack", [128, 32], FP), ("nidstack", [128, 32], FP),
    ):
        d[name] = nc.dram_tensor(name, shape, dt_, kind="ExternalInput")
    y = nc.dram_tensor("y", [ROWS, L], BF, kind="ExternalOutput")

    with TileContext(nc) as tc:
        from contextlib import ExitStack

        def act_silu(out_ap, in_ap, bias_ap, zscratch):
            # silu(z), z = in + bias. sim_safe path avoids the Silu LUT
            # (not implemented in CoreSim): z*sigmoid(z) via ACT+DVE.
            if not sim_safe:
                nc.scalar.activation(out_ap, in_ap, AF.Silu, bias=bias_ap)
            else:
                nc.scalar.activation(out_ap, in_ap, AF.Sigmoid, bias=bias_ap)
                nc.vector.tensor_scalar_add(zscratch, in_ap, bias_ap)
                nc.vector.tensor_mul(out_ap, out_ap, zscratch)

        def emit_body():
            with ExitStack() as ctx:
                const = ctx.enter_context(tc.tile_pool(name="const", bufs=1))
                small = ctx.enter_context(tc.tile_pool(name="small", bufs=1))
                xpool = ctx.enter_context(tc.tile_pool(name="xp", bufs=1))
                xtp = ctx.enter_context(tc.tile_pool(name="xtp", bufs=8))
                sop = ctx.enter_context(tc.tile_pool(name="sop", bufs=6))
                zpool = ctx.enter_context(tc.tile_pool(name="zp", bufs=2))

                # ---- constant loads (small first so the head can start) ----
                def cload(name, shape, dt_=FP):
                    t = const.tile(shape, dt_, tag=name, name=name)
                    nc.sync.dma_start(out=t[:], in_=d[name][:])
                    return t

                # x tiles [128, 1024] x 8 halves per row-group, loaded
                # just-in-time for the fwd sweep: even halves + tail odd
                # halves on the scalar ring, F + early odd halves + the
                # head consts on sync
                id128h_t = cload("id128h", [128, 128], BF)
                xt = [[xpool.tile([128, 1024], BF, tag=f"x{t}{h}", name=f"x{t}{h}")
                       for h in range(8)] for t in range(2)]
                Fq = [const.tile([128, 2048], BF, tag=f"F{q}", name=f"F{q}") for q in range(4)]

                def xload(eng, t, h):
                    eng.dma_start(
                        out=xt[t][h][:],
                        in_=d["x4"][t * 128:(t + 1) * 128, h * 1024:(h + 1) * 1024])

                for h in (0, 2, 4, 6):
                    for t in range(2):
                        xload(nc.scalar, t, h)

                nc.sync.dma_start(out=Fq[0][:], in_=d["F"][:, 0:2048])
                for t in range(2):
                    xload(nc.sync, t, 1)
                embT_t = cload("embT", [128, 2 * B_LOC])
                A_t = cload("A", [128, 256])
                id128_t = cload("id128", [128, 128])
                w1T_t = cload("w1T", [128, 128])
                b1_t = cload("b1", [64, 1])
                w2T_t = cload("w2T", [64, 128])
                nc.sync.dma_start(out=Fq[1][:], in_=d["F"][:, 2048:4096])
                for t in range(2):
                    xload(nc.sync, t, 3)
                nc.sync.dma_start(out=Fq[2][:], in_=d["F"][:, 4096:6144])
                for t in range(2):
                    xload(nc.sync, t, 5)
                nc.sync.dma_start(out=Fq[3][:], in_=d["F"][:, 6144:8192])
                for t in range(2):
                    xload(nc.sync, t, 7)
                b2r_t = cload("b2r", [4, 128])
                lbr_t = cload("lbr", [4, 64])
                lwT2_t = cload("lwT2", [128, 64])
                ones_t = cload("ones", [1, 64])
                ids_t = cload("idstack", [128, 32])
                nids_t = cload("nidstack", [128, 32])

                W_t = const.tile([128, 8192], BF, tag="W", name="W")
                Gq = [const.tile([128, 2048], BF, tag=f"G{q}", name=f"G{q}") for q in range(4)]

                # ---- head: phi, MLP, scaled time weights, folded bias ----
                phi_sb = small.tile([128, B_LOC], FP, tag="phi")
                phi4rep = small.tile([128, 16], FP, tag="phi4rep")
                gbT_sb = small.tile([4, 128], FP, tag="gbT")
                gbrows = small.tile([1, 256], FP, tag="gbrows")
                biasvec = small.tile([4, 64], FP, tag="biasvec")
                bt = [small.tile([128, 1], FP, tag=f"bt{t}", name=f"bt{t}") for t in range(2)]
                linwb2 = [small.tile([128, 128], BF, tag=f"lw{t}", name=f"lw{t}") for t in range(2)]
                tmp44 = small.tile([4, 64], FP, tag="tmp44")

                pf = tc.alloc_tile_pool(name="ps_fwd", bufs=5, space="PSUM")
                prt = tc.alloc_tile_pool(name="ps_rt", bufs=1, space="PSUM")
                with tc.tile_pool(name="ps_head", bufs=2, space="PSUM") as ph:
                    phiT_p = ph.tile([B_LOC, 128], FP, tag="hps", name="phiT_p")
                    for kc in range(2):
                        nc.tensor.matmul(phiT_p[:],
                                         lhsT=embT_t[:, kc * 4:(kc + 1) * 4],
                                         rhs=A_t[:, kc * 128:(kc + 1) * 128],
                                         start=(kc == 0), stop=(kc == 1))
                    phiT_sb = small.tile([B_LOC, 128], FP, tag="phiT_sb")
                    nc.vector.tensor_copy(phiT_sb[:], phiT_p[:])
                    phi_p = ph.tile([128, B_LOC], FP, tag="hps", name="phi_p")
                    nc.tensor.transpose(phi_p[:], phiT_sb[:], id128_t[0:4, 0:4])
                    nc.vector.tensor_copy(phi_sb[:], phi_p[:])
                    # phi4rep[32r+m, dd*8+br*4+b] = phi[dd*64+br*32+m, b]
                    for dd in range(2):
                        for br in range(2):
                            nc.gpsimd.dma_start(
                                out=phi4rep[0:32, dd * 8 + br * 4:dd * 8 + br * 4 + 4],
                                in_=phi_sb[dd * 64 + br * 32:dd * 64 + br * 32 + 32, :])
                    for r in range(1, 4):
                        nc.gpsimd.dma_start(out=phi4rep[32 * r:32 * (r + 1), :],
                                            in_=phi4rep[0:32, :])

                    h_p = ph.tile([HID, B_LOC], FP, tag="hps", name="h_p")
                    for kc in range(2):
                        nc.tensor.matmul(h_p[:],
                                         lhsT=w1T_t[:, kc * 64:(kc + 1) * 64],
                                         rhs=embT_t[:, kc * 4:(kc + 1) * 4],
                                         start=(kc == 0), stop=(kc == 1))
                    h_sb = small.tile([HID, B_LOC], FP, tag="h_sb")
                    hz = small.tile([HID, B_LOC], FP, tag="hz")
                    act_silu(h_sb[:], h_p[:], b1_t[:, 0:1], hz[:])

                    gbT_p = ph.tile([4, 128], FP, tag="hps", name="gbT_p")
                    nc.tensor.matmul(gbT_p[:], lhsT=h_sb[:], rhs=w2T_t[:],
                                     start=True, stop=True)
                    nc.vector.tensor_add(gbT_sb[:], gbT_p[:], b2r_t[:])

                    # biasvec = gamma*lin_b + lin_b + beta
                    nc.vector.tensor_mul(tmp44[:], gbT_sb[:, 0:64], lbr_t[:])
                    nc.vector.tensor_add(tmp44[:], tmp44[:], lbr_t[:])
                    nc.vector.tensor_add(biasvec[:], tmp44[:], gbT_sb[:, 64:128])
                    for t in range(2):
                        for j in range(2):
                            nc.gpsimd.dma_start(
                                out=bt[t][j * 64:(j + 1) * 64, :],
                                in_=biasvec[2 * t + j:2 * t + j + 1, :])

                    # gbrows[0, b*64+o] = gamma[b, o] (partition-0 gather)
                    nc.gpsimd.dma_start(out=gbrows[:], in_=gbT_sb[:, 0:64])
                    # linwb2[t] is block-diagonal [(j,c), (j,o)]:
                    # diag block j = lin_w.T * (1 + gamma[2t+j]) -> the time
                    # branch becomes one K=128 matmul per chunk
                    for t in range(2):
                        rep_p = ph.tile([128, 64], FP, tag="hps", name="rep_p")
                        for j in range(2):
                            b = 2 * t + j
                            nc.tensor.matmul(rep_p[j * 64:(j + 1) * 64, :],
                                             lhsT=ones_t[:],
                                             rhs=gbrows[0:1, b * 64:(b + 1) * 64],
                                             start=True, stop=True)
                        nc.vector.memset(linwb2[t][0:64, 64:128], 0.0)
                        nc.vector.memset(linwb2[t][64:128, 0:64], 0.0)
                        for j in range(2):
                            sl = slice(j * 64, (j + 1) * 64)
                            nc.vector.tensor_mul(linwb2[t][sl, sl], lwT2_t[sl, :],
                                                 rep_p[sl, :])
                            nc.vector.tensor_add(linwb2[t][sl, sl],
                                                 linwb2[t][sl, sl], lwT2_t[sl, :])

                nc.gpsimd.dma_start(out=W_t[:], in_=d["W"][:])
                for q in range(4):
                    nc.gpsimd.dma_start(out=Gq[q][:],
                                        in_=d["G"][:, q * 2048:(q + 1) * 2048])

                # ---- XS with phi folded via diagonal transpose-matmuls ----
                # dtile quadrant (br,din) lives at partition base br*64+din*32;
                # slot (b, dout): din=0 -> {pr, pi}, din=1 -> {-pi, pr}
                dtile = small.tile([128, 256], FP, tag="dtile")
                for br in range(2):
                    for din in range(2):
                        base = br * 64 + din * 32
                        psl = slice(base, base + 32)
                        for b in range(B_LOC):
                            cpr, cpi = br * 4 + b, 8 + br * 4 + b
                            s0 = slice((b * 2) * 32, (b * 2) * 32 + 32)
                            s1 = slice((b * 2 + 1) * 32, (b * 2 + 1) * 32 + 32)
                            if din == 0:
                                nc.vector.tensor_scalar_mul(
                                    dtile[psl, s0], ids_t[psl, :],
                                    phi4rep[psl, cpr:cpr + 1])
                                nc.vector.tensor_scalar_mul(
                                    dtile[psl, s1], ids_t[psl, :],
                                    phi4rep[psl, cpi:cpi + 1])
                            else:
                                nc.vector.tensor_scalar_mul(
                                    dtile[psl, s0], nids_t[psl, :],
                                    phi4rep[psl, cpi:cpi + 1])
                                nc.vector.tensor_scalar_mul(
                                    dtile[psl, s1], ids_t[psl, :],
                                    phi4rep[psl, cpr:cpr + 1])


                # ---- fwd DFT: RT[modecol, rows] ----
                RT_sb = small.tile([128, ROWS], FP, tag="RT")
                if True:
                    rtp = prt.tile([128, ROWS], FP, tag="rtp")
                    # software-pipelined emission: the fwd matmul for chunk
                    # c-3 is emitted after chunk c's transposes so PE never
                    # stalls waiting for the PSUM->SBUF copy
                    LAG = 4
                    xts_l = [None] * 64
                    for c in range(64 + LAG):
                        if c < 64:
                            hh, kk = divmod(c, 8)
                            off = kk * 128
                            tp = pf.tile([128, 256], BF, tag="tp")
                            nc.tensor.transpose(tp[:, 0:128],
                                                xt[0][hh][:, off:off + 128],
                                                id128h_t[:])
                            nc.tensor.transpose(tp[:, 128:256],
                                                xt[1][hh][:, off:off + 128],
                                                id128h_t[:])
                            xts = xtp.tile([128, 256], BF, tag="xts")
                            if (c < 16) or (c < 44 and c % 2 == 0) or \
                                    (c >= 44 and c % 2 == 1):
                                nc.vector.tensor_copy(xts[:], tp[:])
                            else:
                                nc.scalar.copy(xts[:], tp[:])
                            xts_l[c] = xts
                        if c >= LAG:
                            cc = c - LAG
                            qq, kk2 = divmod(cc, 16)
                            off2 = kk2 * 128
                            nc.tensor.matmul(rtp[:],
                                             lhsT=Fq[qq][:, off2:off2 + 128],
                                             rhs=xts_l[cc][:],
                                             start=(cc == 0), stop=(cc == 63))
                            xts_l[cc] = None
                    nc.vector.tensor_copy(RT_sb[:], rtp[:])
                prt.release()
                pf.release()

                XS_sb = [small.tile([128, 128], BF, tag=f"XS{br}",
                                    name=f"XS{br}") for br in range(2)]
                spec_sb = small.tile([128, 256], FP, tag="spec")
                R2f = small.tile([128, ROWS], BF, tag="R2f")
                with tc.tile_pool(name="ps_mid", bufs=1, space="PSUM") as pm:
                    for br in range(2):
                        xsp = pm.tile([128, 128], FP, tag=f"xsp{br}",
                                      name=f"xsp{br}")
                        # regular matmul against the [64, 32] stacked-diagonal
                        # rhs: contracts over (din, m) partitions, summing the
                        # re/im contributions with phi folded in
                        for b in range(B_LOC):
                            psl = slice(br * 64, br * 64 + 64)
                            for dout in range(2):
                                fsl = slice((b * 2 + dout) * 32,
                                            (b * 2 + dout) * 32 + 32)
                                nc.tensor.matmul(
                                    xsp[dout * 64:(dout + 1) * 64, b::4],
                                    lhsT=RT_sb[psl, b * 64:(b + 1) * 64],
                                    rhs=dtile[psl, fsl],
                                    start=True, stop=True)
                        nc.vector.tensor_copy(XS_sb[br][:], xsp[:])

                    # spectral matmuls: M=128 merges both dout blocks
                    spp = pm.tile([128, 256], FP, tag="spp")
                    for br in range(2):
                        for m in range(M):
                            col = (br * 32 + m) * 4
                            nc.tensor.matmul(
                                spp[:, col:col + 4],
                                lhsT=W_t[:, (br * 32 + m) * 128:
                                         (br * 32 + m) * 128 + 128],
                                rhs=XS_sb[br][:, m * 4:(m + 1) * 4],
                                start=True, stop=True)
                    nc.vector.tensor_copy(spec_sb[:], spp[:])

                    # R2 identity-matmul transposes -> R2f [(d,br,m), (b,o)]
                    r2p = pm.tile([128, ROWS], FP, tag="r2p")
                    for dout in range(2):
                        dsl = slice(dout * 64, (dout + 1) * 64)
                        for b in range(B_LOC):
                            nc.tensor.matmul(
                                r2p[dsl, b * 64:(b + 1) * 64],
                                lhsT=spec_sb[dsl, b::4],
                                rhs=id128_t[dsl, dsl],
                                start=True, stop=True)
                    nc.vector.tensor_copy(R2f[:], r2p[:])

                # ---- inverse DFT + time branch + silu + store ----
                with tc.tile_pool(name="ps_out", bufs=2, space="PSUM") as po:
                    for t in range(2):
                        for q in range(4):
                            pos = po.tile([128, 2048], FP, tag="po",
                                          name=f"po{t}{q}")
                            # time branch first: it has no R2f dependency,
                            # so PE prefills the PSUM during the mid-phase
                            # gap; only 4 inverse matmuls gate each silu
                            for kk in range(4):
                                ch = q * 4 + kk
                                nc.tensor.matmul(
                                    pos[:, kk * 512:(kk + 1) * 512],
                                    lhsT=linwb2[t][:],
                                    rhs=xt[t][ch // 2][:, (ch % 2) * 512:
                                                       (ch % 2) * 512 + 512],
                                    start=True, stop=False)
                            for kk in range(4):
                                nc.tensor.matmul(
                                    pos[:, kk * 512:(kk + 1) * 512],
                                    lhsT=R2f[:, t * 128:(t + 1) * 128],
                                    rhs=Gq[q][:, kk * 512:(kk + 1) * 512],
                                    start=False, stop=True)
                            so = sop.tile([128, 2048], BF, tag="so")
                            edge = (t == 0 and q == 0) or (t == 1 and q == 3)
                            if edge:
                                # split edge quarters so the first y DMA
                                # starts earlier / the last overlaps silu
                                for hf in range(2):
                                    sl = slice(hf * 1024, (hf + 1) * 1024)
                                    zs = (zpool.tile([128, 1024], FP, tag="zs",
                                                     name="zs")[:]
                                          if sim_safe else None)
                                    act_silu(so[:, sl], pos[:, sl],
                                             bt[t][:, 0:1], zs)
                                    nc.sync.dma_start(
                                        out=y[t * 128:(t + 1) * 128,
                                              q * 2048 + hf * 1024:
                                              q * 2048 + (hf + 1) * 1024],
                                        in_=so[:, sl])
                            else:
                                zs = (zpool.tile([128, 2048], FP, tag="zs",
                                                 name="zs")[:]
                                      if sim_safe else None)
                                act_silu(so[:], pos[:], bt[t][:, 0:1], zs)
                                nc.sync.dma_start(
                                    out=y[t * 128:(t + 1) * 128,
                                          q * 2048:(q + 1) * 2048],
                                    in_=so[:])

        for _rep in range(nrep):
            emit_body()

    if split:
        _split_multiwait(nc)
    return nc


_NC = None


def _get_nc():
    global _NC
    if _NC is None:
        _NC = _build_nc()
    return _NC


def kernel(**inputs):
    inputs = {k: np.asarray(v) for k, v in inputs.items()}
    x, emb = inputs["x"], inputs["emb"]
    consts = _build_constants(**{k: v for k, v in inputs.items()
                                 if k not in ("x", "emb")})
    nc = _get_nc()

    in_maps = []
    for core in range(N_CORES):
        b0 = core * B_LOC
        m = dict(consts)
        m["x4"] = np.ascontiguousarray(
            x[b0:b0 + B_LOC].reshape(ROWS, L).astype(np.float32), np.float16)
        eT = emb[b0:b0 + B_LOC].T.astype(np.float32)
        m["embT"] = np.ascontiguousarray(eT.reshape(2, 128, B_LOC).transpose(1, 0, 2).reshape(128, 2 * B_LOC))
        in_maps.append(m)

    res = run_bass_kernel_spmd(nc, in_maps, core_ids=list(range(N_CORES)))
    out = np.empty((B, C, L), np.float32)
    for core in range(N_CORES):
        b0 = core * B_LOC
        out[b0:b0 + B_LOC] = res.results[core]["y"].astype(
            np.float32).reshape(B_LOC, C, L)
    return out



# revision 7
# speedup vs baseline: 1.3438x; 1.3438x over previous
"""FNO block (nn_FNOBlock_48962627175213) as a Bass/Tile kernel on 8 trn2 cores.

Math: only 64 complex rfft modes (32 low + 32 high) survive into out_ft, so
rfft/irfft collapse into skinny DFT matmuls against precomputed bases.
Data-parallel over batch: each core takes 4 of the 32 batches (256 rows).

v2 design (vs the transpose-on-chip baseline):
  - x is ALSO staged host-side transposed (xT, fp8) so the forward DFT is a
    straight accumulating matmul -- no PE transposes, no PSUM->SBUF copies.
  - fwd and inverse DFT run as fp8e4 DoubleRow matmuls (2 k-tiles per
    partition, half cycles/col).  The spectral branch contributes ~1e-4 of
    the output magnitude, so fp8 there is numerically free.
  - scale folding: F x64, dtile /128, W x4096, G x8 => spectral PSUM lands
    at 2^14 x true; the time branch matmuls at 2^14 via scaled lin_w; the
    final activation applies scale=2^-14 and the true-scale bias.
  - two batch-groups (2 batches each) pipelined end-to-end so the ACT silu
    pass (the serial bottleneck) starts ~4.5us in, not after the full fwd.
  - out tiles sized [512,1536,2048,1536,2048,512] per group: small first
    tile starts ACT early, small last tile shrinks the store tail; tiles
    alternate between a 4-bank and a 3-bank PSUM pool (+1 bank mid ring).
  - head uses PE transposes / selector matmuls instead of SWDGE gathers.
"""
import sys

if '/opt/trn_rl_repo' not in sys.path:
    sys.path.insert(0, '/opt/trn_rl_repo')

import numpy as np
import ml_dtypes

import concourse.bass as bass
import concourse.mybir as mybir
from concourse.tile import TileContext
from concourse.bass_utils import run_bass_kernel_spmd

FP = mybir.dt.float32
BF = mybir.dt.float16
F8 = mybir.dt.float8e4
E4 = ml_dtypes.float8_e4m3
DR = mybir.MatmulPerfMode.DoubleRow
AF = mybir.ActivationFunctionType

B, C, L, M, EMB, HID = 32, 64, 8192, 32, 256, 64
K = L // 2 + 1
NEG0 = K - M          # 4065
N_CORES = 8
B_LOC = B // N_CORES  # 4
ROWS = B_LOC * C      # 256

SF = 64.0         # F basis scale (fp8)
SD = 1.0 / 128.0  # dtile (phi) scale
SW = 4096.0       # spectral weight scale (fp8)
SG = 8.0          # inverse basis scale (fp8)
ST = 16384.0      # time-branch weight scale == SF*SD*SW*SG (2^14)
DESCALE = 1.0 / ST

# out-tile column sizes per row-group (sum 8192); alternate PSUM pools A/B
SZ = [512, 1536, 2048, 1536, 2048, 512]
OFF = [0, 512, 2048, 4096, 5632, 7680]
NT = len(SZ)


# --------------------------------------------------------------------------
# host-side constant builders
# --------------------------------------------------------------------------
def _build_constants(weights_pos, weights_neg, A_real_pos, A_imag_pos,
                     A_real_neg, A_imag_neg, tm_w1, tm_b1, tm_w2, tm_b2,
                     lin_w, lin_b):
    n = np.arange(L, dtype=np.float64)
    s = 1.0 / np.sqrt(L)

    # fwd DFT basis [8192, 128], col = br*64 + m (cos) / br*64+32+m (-sin)
    F = np.zeros((L, 128), np.float64)
    for br in range(2):
        for m in range(M):
            k = m if br == 0 else NEG0 + m
            ang = 2.0 * np.pi * k * n / L
            F[:, br * 64 + m] = np.cos(ang) * s
            F[:, br * 64 + 32 + m] = -np.sin(ang) * s
    # DoubleRow layout [128 p, 2 j, 32 c, 128 mode]: F_dr[p,j,c,m]=F[(2c+j)*128+p, m]
    F_dr = (F * SF).reshape(32, 2, 128, 128).transpose(2, 1, 0, 3)
    F_dr = np.ascontiguousarray(F_dr).astype(E4)

    # inverse basis [128, 8192], row = d*64 + br*32 + m (pocketfft irfft
    # semantics: Im parts of DC and Nyquist are discarded)
    G = np.zeros((128, L), np.float64)
    for br in range(2):
        for m in range(M):
            k = m if br == 0 else NEG0 + m
            ang = 2.0 * np.pi * k * n / L
            if k == 0:
                G[br * 32 + m] = s
            elif k == L // 2:
                G[br * 32 + m] = np.cos(np.pi * n) * s
            else:
                G[br * 32 + m] = 2.0 * np.cos(ang) * s
                G[64 + br * 32 + m] = -2.0 * np.sin(ang) * s
    # DoubleRow layout [64 p=(br,m), 2 j=d, 8192]
    G_dr = (G * SG).reshape(2, 64, L).transpose(1, 0, 2)
    G_dr = np.ascontiguousarray(G_dr).astype(E4)

    # spectral weights [128, 8192]: col = ((br*32+m)*2+dout)*64 + o,
    # rows = (din, i); dout=0 -> [wr; -wi], dout=1 -> [wi; wr]
    Wspec = np.zeros((128, 8192), np.float32)
    for br, wfull in ((0, weights_pos), (1, weights_neg)):
        for m in range(M):
            wr = wfull[:, :, m, 0]
            wi = wfull[:, :, m, 1]
            c0 = (br * 32 + m) * 128
            c1 = (br * 32 + m) * 128 + 64
            Wspec[0:64, c0:c0 + 64] = wr
            Wspec[64:128, c0:c0 + 64] = -wi
            Wspec[0:64, c1:c1 + 64] = wi
            Wspec[64:128, c1:c1 + 64] = wr
    Wspec = (Wspec * SW).astype(E4)

    # phi projector [256 emb, 256]: cols 0:128 = slot-A layout, 128:256 =
    # slot-B.  Col p = br*64 + din*32 + m matches the dtile partition:
    # A: din0 -> re[br], din1 -> im[br]; B: din0 -> im[br], din1 -> re[br].
    re_ = {0: A_real_pos.T, 1: A_real_neg.T}
    im_ = {0: A_imag_pos.T, 1: A_imag_neg.T}
    Astack = np.zeros((EMB, 256), np.float32)
    for br in range(2):
        Astack[:, br * 64:br * 64 + 32] = re_[br]
        Astack[:, br * 64 + 32:br * 64 + 64] = im_[br]
        Astack[:, 128 + br * 64:128 + br * 64 + 32] = im_[br]
        Astack[:, 128 + br * 64 + 32:128 + br * 64 + 64] = re_[br]
    # k-chunk repack [128, 2*256] (SBUF tiles cap at 128 partitions)
    Astack = np.ascontiguousarray(
        Astack.reshape(2, 128, 256).transpose(1, 0, 2).reshape(128, 512))

    w1T = tm_w1.T.astype(np.float32)  # [256, 64] -> [128, 2*64]
    w1T = np.ascontiguousarray(
        w1T.reshape(2, 128, 64).transpose(1, 0, 2).reshape(128, 128))

    # batch selector for gamma broadcast: selt[p, t*128 + j*64 + c] = (p==2t+j)
    selt = np.zeros((4, 256), np.float32)
    for t in range(2):
        for j in range(2):
            selt[2 * t + j, t * 128 + j * 64:t * 128 + (j + 1) * 64] = 1.0
    # bias selector: cols j*2+t pick batch 2t+j
    bsel = np.zeros((4, 4), np.float32)
    for j in range(2):
        for t in range(2):
            bsel[2 * t + j, j * 2 + t] = 1.0

    out = dict(
        F0=np.ascontiguousarray(F_dr[:, :, 0:16, :]),
        F1=np.ascontiguousarray(F_dr[:, :, 16:32, :]),
        A=Astack,
        w1T=w1T,
        b1=np.ascontiguousarray(tm_b1[:, None], np.float32),
        w2T=np.ascontiguousarray(tm_w2.T, np.float32),
        b2r=np.ascontiguousarray(np.tile(tm_b2, (4, 1)), np.float32),
        lbr=np.ascontiguousarray(np.tile(lin_b, (4, 1)), np.float32),
        lwT2=np.ascontiguousarray(np.tile(lin_w.T * ST, (2, 1)), np.float32),
        selt=selt,
        bsel=bsel,
        id4=np.eye(4, dtype=np.float32),
        id128h=np.eye(128, dtype=np.float16),
        idstack=np.ascontiguousarray(
            np.tile(np.eye(32) * SD, (4, 1)), np.float16),
        nidstack=np.ascontiguousarray(
            np.tile(-np.eye(32) * SD, (4, 1)), np.float16),
    )
    for q in range(4):
        out[f"Wq{q}"] = np.ascontiguousarray(Wspec[:, q * 2048:(q + 1) * 2048])
    for k in range(NT):
        out[f"Gt{k}"] = np.ascontiguousarray(
            G_dr[:, :, OFF[k]:OFF[k] + SZ[k]])
    return out


def _stage_x(x_loc):
    """per-core x staging: fp16 row-major + fp8 DoubleRow-transposed."""
    xf = x_loc.reshape(ROWS, L).astype(np.float32)
    x16 = np.ascontiguousarray(xf, np.float16)
    # xT_dr[p, j, c, row] = x[row, (2c+j)*128 + p], split by row-group
    xT = xf.T.reshape(32, 2, 128, ROWS).transpose(2, 1, 0, 3)  # [128,2,32,256]
    out = {"x4": x16}
    for t in range(2):
        rows = slice(t * 128, (t + 1) * 128)
        out[f"xT{t}0"] = np.ascontiguousarray(xT[:, :, 0:16, rows]).astype(E4)
        out[f"xT{t}1"] = np.ascontiguousarray(xT[:, :, 16:32, rows]).astype(E4)
    return out


# --------------------------------------------------------------------------
# walrus workaround: this container's walrus rejects >1 sync-wait on
# TPB_CTRL lowering (Drain/NoOp). Split extra waits onto preceding NOPs.
# --------------------------------------------------------------------------
def _split_multiwait(nc, max_waits=1):
    for f in nc.m.functions:
        for blk in f.blocks:
            new = []
            changed = False
            for inst in blk.instructions:
                si = inst.sync_info
                if (si is not None and len(si.on_wait) > max_waits):
                    waits = list(si.on_wait)
                    head, tail = waits[:-max_waits], waits[-max_waits:]
                    for j, w in enumerate(head):
                        nop = mybir.InstNoOp(name=f"{inst.name}-ws{j}",
                                             ins=[], outs=[])
                        nop.engine = inst.engine
                        nop.sync_info = mybir.SyncInfo(on_wait=[w], on_update=[])
                        new.append(nop)
                    inst.sync_info = mybir.SyncInfo(on_wait=tail,
                                                    on_update=list(si.on_update))
                    changed = True
                new.append(inst)
            if changed:
                blk.instructions = new


# --------------------------------------------------------------------------
# the bass program (input-value independent; built once)
# --------------------------------------------------------------------------
def _build_nc(split=True):
    nc = bass.Bass("TRN2")
    d = {}
    specs = [
        ("x4", [ROWS, L], BF), ("embT", [128, 2 * B_LOC], FP),
        ("xT00", [128, 2, 16, 128], F8), ("xT01", [128, 2, 16, 128], F8),
        ("xT10", [128, 2, 16, 128], F8), ("xT11", [128, 2, 16, 128], F8),
        ("F0", [128, 2, 16, 128], F8), ("F1", [128, 2, 16, 128], F8),
        ("A", [128, 512], FP),
        ("w1T", [128, 128], FP), ("b1", [64, 1], FP), ("w2T", [64, 128], FP),
        ("b2r", [4, 128], FP), ("lbr", [4, 64], FP), ("lwT2", [128, 64], FP),
        ("selt", [4, 256], FP), ("bsel", [4, 4], FP), ("id4", [4, 4], FP),
        ("id128h", [128, 128], BF),
        ("idstack", [128, 32], BF), ("nidstack", [128, 32], BF),
    ]
    for q in range(4):
        specs.append((f"Wq{q}", [128, 2048], F8))
    for k in range(NT):
        specs.append((f"Gt{k}", [64, 2, SZ[k]], F8))
    for name, shape, dt_ in specs:
        d[name] = nc.dram_tensor(name, shape, dt_, kind="ExternalInput")
    y = nc.dram_tensor("y", [ROWS, L], BF, kind="ExternalOutput")

    with TileContext(nc) as tc:
        from contextlib import ExitStack
        with ExitStack() as ctx:
            const = ctx.enter_context(tc.tile_pool(name="const", bufs=1))
            small = ctx.enter_context(tc.tile_pool(name="small", bufs=1))
            sop = ctx.enter_context(tc.tile_pool(name="sop", bufs=8))

            def cload(eng, name, shape, dt_=FP):
                t = const.tile(shape, dt_, tag=name, name=name)
                eng.dma_start(out=t[:], in_=d[name][:])
                return t

            # ---- ACT queue: head consts, W first half ----
            embT_t = cload(nc.scalar, "embT", [128, 2 * B_LOC])
            A_t = cload(nc.scalar, "A", [128, 512])
            id4_t = cload(nc.scalar, "id4", [4, 4])
            w1T_t = cload(nc.scalar, "w1T", [128, 128])
            b1_t = cload(nc.scalar, "b1", [64, 1])
            ids_t = cload(nc.scalar, "idstack", [128, 32], BF)
            nids_t = cload(nc.scalar, "nidstack", [128, 32], BF)
            w2T_t = cload(nc.scalar, "w2T", [64, 128])
            b2r_t = cload(nc.scalar, "b2r", [4, 128])
            lbr_t = cload(nc.scalar, "lbr", [4, 64])
            lwT2_t = cload(nc.scalar, "lwT2", [128, 64])
            selt_t = cload(nc.scalar, "selt", [4, 256])
            bsel_t = cload(nc.scalar, "bsel", [4, 4])
            id128h_t = cload(nc.scalar, "id128h", [128, 128], BF)
            Wq = [None] * 4
            Wq[0] = cload(nc.scalar, "Wq0", [128, 2048], F8)
            Wq[1] = cload(nc.scalar, "Wq1", [128, 2048], F8)

            # ---- SP queue: F, Wq2, bulk of x4 ----
            Fh = [cload(nc.sync, f"F{h}", [128, 2, 16, 128], F8)
                  for h in range(2)]
            Wq[2] = cload(nc.sync, "Wq2", [128, 2048], F8)
            x4t = [[const.tile([128, SZ[k]], BF, tag=f"x4_{t}{k}",
                               name=f"x4_{t}{k}") for k in range(NT)]
                   for t in range(2)]

            def x4load(eng, t, k):
                eng.dma_start(
                    out=x4t[t][k][:],
                    in_=d["x4"][t * 128:(t + 1) * 128,
                                OFF[k]:OFF[k] + SZ[k]])

            for k in range(2, NT):
                x4load(nc.sync, 0, k)
            for k in range(0, 3):
                x4load(nc.sync, 1, k)

            # ---- Pool queue: xT, Wq3, G tiles, tail of x4 ----
            xTg = [[const.tile([128, 2, 16, 128], F8, tag=f"xT{t}{h}",
                               name=f"xT{t}{h}") for h in range(2)]
                   for t in range(2)]
            Gt = [const.tile([64, 2, SZ[k]], F8, tag=f"Gt{k}", name=f"Gt{k}")
                  for k in range(NT)]
            Wq[3] = const.tile([128, 2048], F8, tag="Wq3", name="Wq3")

            def pload(t, name):
                nc.gpsimd.dma_start(out=t[:], in_=d[name][:])

            pload(xTg[0][0], "xT00")
            pload(xTg[0][1], "xT01")
            pload(Wq[3], "Wq3")
            pload(Gt[0], "Gt0")
            pload(Gt[1], "Gt1")
            pload(Gt[2], "Gt2")
            pload(xTg[1][0], "xT10")
            pload(Gt[3], "Gt3")
            pload(xTg[1][1], "xT11")
            pload(Gt[4], "Gt4")
            pload(Gt[5], "Gt5")
            for k in range(3, NT):
                x4load(nc.gpsimd, 1, k)

            # ---- head A: phi -> dtile, h (MLP layer 1) ----
            phiT_sb = small.tile([B_LOC, 256], FP, tag="phiT_sb")
            phiAB = small.tile([128, 8], FP, tag="phiAB")
            h_sb = small.tile([HID, B_LOC], FP, tag="h_sb")
            gbT_sb = small.tile([4, 128], FP, tag="gbT")
            biasvec = small.tile([4, 64], FP, tag="biasvec")
            bt_sb = small.tile([128, 2], FP, tag="bt_sb")
            linwb2 = [small.tile([128, 128], BF, tag=f"lw{t}", name=f"lw{t}")
                      for t in range(2)]
            tmp44 = small.tile([4, 64], FP, tag="tmp44")
            dtile = small.tile([128, 256], BF, tag="dtile")

            pm = tc.alloc_tile_pool(name="ps_mid", bufs=1, space="PSUM")
            ph = tc.alloc_tile_pool(name="ps_head", bufs=2, space="PSUM")
            phiT_p = ph.tile([B_LOC, 256], FP, tag="hps", name="phiT_p")
            for kc in range(2):
                nc.tensor.matmul(phiT_p[:],
                                 lhsT=embT_t[:, kc * 4:(kc + 1) * 4],
                                 rhs=A_t[:, kc * 256:(kc + 1) * 256],
                                 start=(kc == 0), stop=(kc == 1))
            nc.vector.tensor_copy(phiT_sb[:], phiT_p[:])

            # phiAB[p, 0:4] = slot-A phi components, [p, 4:8] = slot-B,
            # already in dtile partition layout (2 PE transposes)
            pab = ph.tile([128, 8], FP, tag="hps", name="pab")
            for i in range(2):
                nc.tensor.transpose(pab[:, i * 4:(i + 1) * 4],
                                    phiT_sb[:, i * 128:(i + 1) * 128],
                                    id4_t[:])
            nc.vector.tensor_copy(phiAB[:], pab[:])

            # dtile[(br,din,m), (b,s)*32 block] = +/- eye/128 * phi component
            for br in range(2):
                for din in range(2):
                    base = br * 64 + din * 32
                    psl = slice(base, base + 32)
                    eyeA = ids_t if din == 0 else nids_t
                    for b in range(B_LOC):
                        s0 = slice((b * 2) * 32, (b * 2) * 32 + 32)
                        s1 = slice((b * 2 + 1) * 32, (b * 2 + 1) * 32 + 32)
                        nc.vector.tensor_scalar_mul(
                            dtile[psl, s0], eyeA[psl, :], phiAB[psl, b:b + 1])
                        nc.vector.tensor_scalar_mul(
                            dtile[psl, s1], ids_t[psl, :],
                            phiAB[psl, 4 + b:5 + b])

            h_p = ph.tile([HID, B_LOC], FP, tag="hps", name="h_p")
            for kc in range(2):
                nc.tensor.matmul(h_p[:],
                                 lhsT=w1T_t[:, kc * 64:(kc + 1) * 64],
                                 rhs=embT_t[:, kc * 4:(kc + 1) * 4],
                                 start=(kc == 0), stop=(kc == 1))
            nc.scalar.activation(h_sb[:], h_p[:], AF.Silu, bias=b1_t[:, 0:1])

            # first two x4 tiles ride the ACT queue after the head silu
            x4load(nc.scalar, 0, 0)
            x4load(nc.scalar, 0, 1)

            # ---- mid-pipeline state + pools ----
            RT_sb = [small.tile([128, 128], BF, tag=f"RT{t}", name=f"RT{t}")
                     for t in range(2)]
            XS_sb = [[small.tile([128, 64], BF, tag=f"XS{t}{br}",
                                 name=f"XS{t}{br}") for br in range(2)]
                     for t in range(2)]
            spec_sb = [small.tile([128, 128], BF, tag=f"spec{t}",
                                  name=f"spec{t}") for t in range(2)]
            R2f = [small.tile([64, 2, 128], F8, tag=f"R2f{t}", name=f"R2f{t}")
                   for t in range(2)]

            def fwd_mid(t):
                # fwd DFT: 32 DoubleRow matmuls, K=256 per matmul
                rtp = pm.tile([128, 128], FP, tag="mid", name=f"rtp{t}")
                for c in range(32):
                    hh, cc = divmod(c, 16)
                    nc.tensor.matmul(rtp[:],
                                     lhsT=Fh[hh][:, :, cc, :],
                                     rhs=xTg[t][hh][:, :, cc, :],
                                     start=(c == 0), stop=(c == 31),
                                     perf_mode=DR)
                nc.vector.tensor_copy(RT_sb[t][:], rtp[:])

                # XS: fold phi via stacked-diagonal rhs
                for br in range(2):
                    xsp = pm.tile([128, 64], FP, tag="mid", name=f"xsp{t}{br}")
                    psl = slice(br * 64, br * 64 + 64)
                    for j in range(2):
                        b = 2 * t + j
                        for dout in range(2):
                            fsl = slice((b * 2 + dout) * 32,
                                        (b * 2 + dout) * 32 + 32)
                            nc.tensor.matmul(
                                xsp[dout * 64:(dout + 1) * 64, j::2],
                                lhsT=RT_sb[t][psl, j * 64:(j + 1) * 64],
                                rhs=dtile[psl, fsl],
                                start=True, stop=True)
                    nc.vector.tensor_copy(XS_sb[t][br][:], xsp[:])

                # spectral: per-mode matmuls, N=2 (this group's 2 batches)
                spp = pm.tile([128, 128], FP, tag="mid", name=f"spp{t}")
                for br in range(2):
                    for m in range(M):
                        wt = Wq[br * 2 + m // 16]
                        col = (m % 16) * 128
                        nc.tensor.matmul(
                            spp[:, (br * 32 + m) * 2:(br * 32 + m) * 2 + 2],
                            lhsT=wt[:, col:col + 128],
                            rhs=XS_sb[t][br][:, m * 2:(m + 1) * 2],
                            start=True, stop=True)
                nc.vector.tensor_copy(spec_sb[t][:], spp[:])

                # R2 transposes -> R2f_dr [64 p=(br,m), 2 j=dout, 128 (j,o)]
                r2p = pm.tile([64, 256], FP, tag="mid", name=f"r2p{t}")
                for dout in range(2):
                    dsl = slice(dout * 64, (dout + 1) * 64)
                    for j in range(2):
                        nc.tensor.matmul(
                            r2p[0:64, dout * 128 + j * 64:
                                dout * 128 + (j + 1) * 64],
                            lhsT=spec_sb[t][dsl, j::2],
                            rhs=id128h_t[dsl, dsl],
                            start=True, stop=True)
                nc.vector.tensor_copy(R2f[t][:], r2p[:])

            fwd_mid(0)

            # ---- head B: gbT, bias vector, scaled time weights ----
            gbT_p = ph.tile([4, 128], FP, tag="hps", name="gbT_p")
            nc.tensor.matmul(gbT_p[:], lhsT=h_sb[:], rhs=w2T_t[:],
                             start=True, stop=True)
            nc.vector.tensor_add(gbT_sb[:], gbT_p[:], b2r_t[:])
            rep_p = [ph.tile([128, 64], FP, tag="hps", name=f"rep{t}")
                     for t in range(2)]
            for t in range(2):
                nc.tensor.matmul(rep_p[t][:],
                                 lhsT=selt_t[:, t * 128:(t + 1) * 128],
                                 rhs=gbT_sb[:, 0:64], start=True, stop=True)
            # biasvec = gamma*lin_b + lin_b + beta (true scale)
            nc.vector.tensor_mul(tmp44[:], gbT_sb[:, 0:64], lbr_t[:])
            nc.vector.tensor_add(tmp44[:], tmp44[:], lbr_t[:])
            nc.vector.tensor_add(biasvec[:], tmp44[:], gbT_sb[:, 64:128])
            # bt_sb[(j,o), t] = biasvec[2t+j, o] via 2 selector matmuls
            btp = ph.tile([128, 2], FP, tag="hps", name="btp")
            for j in range(2):
                nc.tensor.matmul(btp[j * 64:(j + 1) * 64, :],
                                 lhsT=biasvec[:],
                                 rhs=bsel_t[:, j * 2:(j + 1) * 2],
                                 start=True, stop=True)
            nc.vector.tensor_copy(bt_sb[:], btp[:])
            # linwb2[t][(j,c),(j,o)] block-diag = lin_w.T*ST*(1+gamma[2t+j])
            for t in range(2):
                nc.vector.memset(linwb2[t][0:64, 64:128], 0.0)
                nc.vector.memset(linwb2[t][64:128, 0:64], 0.0)
                for j in range(2):
                    sl = slice(j * 64, (j + 1) * 64)
                    nc.vector.tensor_mul(linwb2[t][sl, sl], lwT2_t[sl, :],
                                         rep_p[t][sl, :])
                    nc.vector.tensor_add(linwb2[t][sl, sl],
                                         linwb2[t][sl, sl], lwT2_t[sl, :])
            ph.release()

            poA = tc.alloc_tile_pool(name="ps_oa", bufs=1, space="PSUM")
            poB = tc.alloc_tile_pool(name="ps_ob", bufs=1, space="PSUM")

            def out_tile(t, k):
                po = poA if k % 2 == 0 else poB
                sz = SZ[k]
                nch = sz // 512
                pos = po.tile([128, sz], FP, tag="po", name=f"po{t}{k}")
                for i in range(nch):
                    nc.tensor.matmul(
                        pos[:, i * 512:(i + 1) * 512],
                        lhsT=linwb2[t][:],
                        rhs=x4t[t][k][:, i * 512:(i + 1) * 512],
                        start=True, stop=False)
                for i in range(nch):
                    nc.tensor.matmul(
                        pos[:, i * 512:(i + 1) * 512],
                        lhsT=R2f[t][:],
                        rhs=Gt[k][:, :, i * 512:(i + 1) * 512],
                        start=False, stop=True, perf_mode=DR)
                so = sop.tile([128, sz], BF, tag="so")
                nc.scalar.activation(so[:], pos[:], AF.Silu,
                                     bias=bt_sb[:, t:t + 1], scale=DESCALE)
                eng = nc.gpsimd if t == 0 else nc.sync
                eng.dma_start(
                    out=y[t * 128:(t + 1) * 128, OFF[k]:OFF[k] + sz],
                    in_=so[:])

            for k in range(4):
                out_tile(0, k)
            fwd_mid(1)
            for k in range(4, NT):
                out_tile(0, k)
            for k in range(NT):
                out_tile(1, k)
            poB.release()
            poA.release()
            pm.release()

    if split:
        _split_multiwait(nc)
    return nc


_NC = None


def _get_nc():
    global _NC
    if _NC is None:
        _NC = _build_nc()
    return _NC


def _core_inputs(x, emb, consts, core):
    b0 = core * B_LOC
    m = dict(consts)
    m.update(_stage_x(np.ascontiguousarray(x[b0:b0 + B_LOC])))
    eT = emb[b0:b0 + B_LOC].T.astype(np.float32)
    m["embT"] = np.ascontiguousarray(
        eT.reshape(2, 128, B_LOC).transpose(1, 0, 2).reshape(128, 2 * B_LOC))
    return m


def kernel(**inputs):
    inputs = {k: np.asarray(v) for k, v in inputs.items()}
    x, emb = inputs["x"], inputs["emb"]
    consts = _build_constants(**{k: v for k, v in inputs.items()
                                 if k not in ("x", "emb")})
    nc = _get_nc()

    in_maps = [_core_inputs(x, emb, consts, core) for core in range(N_CORES)]
    res = run_bass_kernel_spmd(nc, in_maps, core_ids=list(range(N_CORES)))
    out = np.empty((B, C, L), np.float32)
    for core in range(N_CORES):
        b0 = core * B_LOC
        out[b0:b0 + B_LOC] = res.results[core]["y"].astype(
            np.float32).reshape(B_LOC, C, L)
    return out


# revision 29
# speedup vs baseline: 1.6466x; 1.2254x over previous
"""FNO block (nn_FNOBlock_48962627175213) as a Bass/Tile kernel on 8 trn2 cores.

Math: only 64 complex rfft modes (32 low + 32 high) survive into out_ft, so
rfft/irfft collapse into skinny DFT matmuls against precomputed bases.
Data-parallel over batch: each core takes 4 of the 32 batches (256 rows).

v2 design (vs the transpose-on-chip baseline):
  - x is ALSO staged host-side transposed (xT, fp8) so the forward DFT is a
    straight accumulating matmul -- no PE transposes, no PSUM->SBUF copies.
  - fwd and inverse DFT run as fp8e4 DoubleRow matmuls (2 k-tiles per
    partition, half cycles/col).  The spectral branch contributes ~1e-4 of
    the output magnitude, so fp8 there is numerically free.
  - scale folding: F x64, dtile /128, W x4096, G x8 => spectral PSUM lands
    at 2^14 x true; the time branch matmuls at 2^14 via scaled lin_w; the
    final activation applies scale=2^-14 and the true-scale bias.
  - two batch-groups (2 batches each) pipelined end-to-end so the ACT silu
    pass (the serial bottleneck) starts ~4.5us in, not after the full fwd.
  - out tiles sized [512,1536,2048,1536,2048,512] per group: small first
    tile starts ACT early, small last tile shrinks the store tail; tiles
    alternate between a 4-bank and a 3-bank PSUM pool (+1 bank mid ring).
  - head uses PE transposes / selector matmuls instead of SWDGE gathers.
"""
import sys

if '/opt/trn_rl_repo' not in sys.path:
    sys.path.insert(0, '/opt/trn_rl_repo')

import numpy as np
import ml_dtypes

import concourse.bass as bass
import concourse.mybir as mybir
from concourse.tile import TileContext
from concourse.bass_utils import run_bass_kernel_spmd

FP = mybir.dt.float32
BF = mybir.dt.float16
F8 = mybir.dt.float8e4
E4 = ml_dtypes.float8_e4m3
DR = mybir.MatmulPerfMode.DoubleRow
AF = mybir.ActivationFunctionType

B, C, L, M, EMB, HID = 32, 64, 8192, 32, 256, 64
K = L // 2 + 1
NEG0 = K - M          # 4065
N_CORES = 8
B_LOC = B // N_CORES  # 4
ROWS = B_LOC * C      # 256

SF = 64.0         # F basis scale (fp8)
SD = 1.0 / 128.0  # dtile (phi) scale
SW = 4096.0       # spectral weight scale (fp8)
SG = 8.0          # inverse basis scale (fp8)
ST = 16384.0      # time-branch weight scale == SF*SD*SW*SG (2^14)
DESCALE = 1.0 / ST

# out-tile column sizes per row-group (sum 8192); alternate PSUM pools A/B
SZ = [512, 1536, 2048, 1536, 2048, 512]
OFF = [0, 512, 2048, 4096, 5632, 7680]
NT = len(SZ)


# --------------------------------------------------------------------------
# host-side constant builders
# --------------------------------------------------------------------------
def _build_constants(weights_pos, weights_neg, A_real_pos, A_imag_pos,
                     A_real_neg, A_imag_neg, tm_w1, tm_b1, tm_w2, tm_b2,
                     lin_w, lin_b):
    n = np.arange(L, dtype=np.float64)
    s = 1.0 / np.sqrt(L)

    # fwd DFT basis [8192, 128], col = br*64 + m (cos) / br*64+32+m (-sin)
    F = np.zeros((L, 128), np.float64)
    for br in range(2):
        for m in range(M):
            k = m if br == 0 else NEG0 + m
            ang = 2.0 * np.pi * k * n / L
            F[:, br * 64 + m] = np.cos(ang) * s
            F[:, br * 64 + 32 + m] = -np.sin(ang) * s
    # DoubleRow layout [128 p, 2 j, 32 c, 128 mode]: F_dr[p,j,c,m]=F[(2c+j)*128+p, m]
    F_dr = (F * SF).reshape(32, 2, 128, 128).transpose(2, 1, 0, 3)
    F_dr = np.ascontiguousarray(F_dr).astype(E4)

    # inverse basis [128, 8192], row = d*64 + br*32 + m (pocketfft irfft
    # semantics: Im parts of DC and Nyquist are discarded)
    G = np.zeros((128, L), np.float64)
    for br in range(2):
        for m in range(M):
            k = m if br == 0 else NEG0 + m
            ang = 2.0 * np.pi * k * n / L
            if k == 0:
                G[br * 32 + m] = s
            elif k == L // 2:
                G[br * 32 + m] = np.cos(np.pi * n) * s
            else:
                G[br * 32 + m] = 2.0 * np.cos(ang) * s
                G[64 + br * 32 + m] = -2.0 * np.sin(ang) * s
    # DoubleRow layout [64 p=(br,m), 2 j=d, 8192]
    G_dr = (G * SG).reshape(2, 64, L).transpose(1, 0, 2)
    G_dr = np.ascontiguousarray(G_dr).astype(E4)

    # spectral weights split by output half so spectral matmuls land at
    # partition base 0: Wd[dout] [128 rows=(din,i), (br*32+m)*64 + o];
    # dout=0 -> [wr; -wi], dout=1 -> [wi; wr]
    Wd = np.zeros((2, 128, 4096), np.float32)
    for br, wfull in ((0, weights_pos), (1, weights_neg)):
        for m in range(M):
            wr = wfull[:, :, m, 0]
            wi = wfull[:, :, m, 1]
            c = (br * 32 + m) * 64
            Wd[0, 0:64, c:c + 64] = wr
            Wd[0, 64:128, c:c + 64] = -wi
            Wd[1, 0:64, c:c + 64] = wi
            Wd[1, 64:128, c:c + 64] = wr
    Wd = (Wd * SW).astype(E4)

    # phi projector [256 emb, 128]: cols 0:64 = re at (br,m), 64:128 = im.
    # phi now applies POST-spectral (it commutes with the channel mix), as
    # a complex rotation on the r2p tile whose partitions are (br,m).
    Astack = np.zeros((EMB, 128), np.float32)
    Astack[:, 0:32] = A_real_pos.T
    Astack[:, 32:64] = A_real_neg.T
    Astack[:, 64:96] = A_imag_pos.T
    Astack[:, 96:128] = A_imag_neg.T
    # k-chunk repack [128, 2*128] (SBUF tiles cap at 128 partitions)
    Astack = np.ascontiguousarray(
        Astack.reshape(2, 128, 128).transpose(1, 0, 2).reshape(128, 256))

    w1T = tm_w1.T.astype(np.float32)  # [256, 64] -> [128, 2*64]
    w1T = np.ascontiguousarray(
        w1T.reshape(2, 128, 64).transpose(1, 0, 2).reshape(128, 128))

    # batch selector for gamma broadcast: selt[p, t*128 + j*64 + c] = (p==2t+j)
    selt = np.zeros((4, 256), np.float32)
    for t in range(2):
        for j in range(2):
            selt[2 * t + j, t * 128 + j * 64:t * 128 + (j + 1) * 64] = 1.0
    # bias selector: cols j*2+t pick batch 2t+j
    bsel = np.zeros((4, 4), np.float32)
    for j in range(2):
        for t in range(2):
            bsel[2 * t + j, j * 2 + t] = 1.0

    # all small consts packed into one fp16 [128, 1425] tensor (1 DMA):
    # cols: embT 0:8 (per-core), A 8:520, w1T 520:648, lwT2 648:712,
    # b1 712:713 (rows 0:64), w2T 713:841 (rows 0:64), b2r 841:969 (rows
    # 0:4), lbr 969:1033, selt 1033:1289, bsel 1289:1293, id4 1293:1297,
    # idstack 1297:1329, nidstack 1329:1361, id64h 1361:1425
    CA = np.zeros((128, 1425), np.float32)
    CA[:, 8:264] = Astack
    # phi-free dtile const [128, 64]: rows (br,din,m), col block dout:
    # eye/128 iff din == dout (pure re/im layout shuffle for XS)
    for br in range(2):
        for dn in range(2):
            CA[br * 64 + dn * 32:br * 64 + dn * 32 + 32,
               264 + dn * 32:264 + (dn + 1) * 32] = np.eye(32) * SD
    CA[:, 520:648] = w1T
    CA[:, 648:712] = np.tile(lin_w.T * ST, (2, 1))
    CA[0:64, 712] = tm_b1
    CA[0:64, 713:841] = tm_w2.T
    CA[0:4, 841:969] = np.tile(tm_b2, (4, 1))
    CA[0:4, 969:1033] = np.tile(lin_b, (4, 1))
    CA[0:4, 1033:1289] = selt
    CA[4, 1033:1289] = 1.0  # ones row: selector matmul yields 1+gamma
    CA[0:4, 1289:1293] = bsel
    CA[0:4, 1293:1297] = np.eye(4)
    CA[:, 1297:1329] = np.tile(np.eye(32) * SD, (4, 1))
    CA[:, 1329:1361] = np.tile(-np.eye(32) * SD, (4, 1))
    CA[0:64, 1361:1425] = np.eye(64)
    out = dict(
        F0=np.ascontiguousarray(F_dr[:, :, 0:16, :]),
        F1=np.ascontiguousarray(F_dr[:, :, 16:32, :]),
        CA=CA.astype(np.float16),
        G=np.ascontiguousarray(G_dr),
    )
    out["Wd0"] = np.ascontiguousarray(Wd[0])
    out["Wq2"] = np.ascontiguousarray(Wd[1][:, 0:2048])
    out["Wq3"] = np.ascontiguousarray(Wd[1][:, 2048:4096])
    return out


def _stage_x(x_loc):
    """per-core x staging: fp16 row-major + fp8 DoubleRow-transposed."""
    xf = x_loc.reshape(ROWS, L).astype(np.float32)
    x16 = np.ascontiguousarray(xf, np.float16)
    # xT_dr[p, j, c, row] = x[row, (2c+j)*128 + p], split by row-group
    xT = xf.T.reshape(32, 2, 128, ROWS).transpose(2, 1, 0, 3)  # [128,2,32,256]
    out = {"x4": x16}
    for t in range(2):
        rows = slice(t * 128, (t + 1) * 128)
        out[f"xT{t}0"] = np.ascontiguousarray(xT[:, :, 0:16, rows]).astype(E4)
        out[f"xT{t}1"] = np.ascontiguousarray(xT[:, :, 16:32, rows]).astype(E4)
    return out


# --------------------------------------------------------------------------
# walrus workaround: this container's walrus rejects >1 sync-wait on
# TPB_CTRL lowering (Drain/NoOp). Split extra waits onto preceding NOPs.
# --------------------------------------------------------------------------
def _split_multiwait(nc, max_waits=1):
    for f in nc.m.functions:
        for blk in f.blocks:
            new = []
            changed = False
            for inst in blk.instructions:
                si = inst.sync_info
                if (si is not None and len(si.on_wait) > max_waits):
                    waits = list(si.on_wait)
                    head, tail = waits[:-max_waits], waits[-max_waits:]
                    for j, w in enumerate(head):
                        nop = mybir.InstNoOp(name=f"{inst.name}-ws{j}",
                                             ins=[], outs=[])
                        nop.engine = inst.engine
                        nop.sync_info = mybir.SyncInfo(on_wait=[w], on_update=[])
                        new.append(nop)
                    inst.sync_info = mybir.SyncInfo(on_wait=tail,
                                                    on_update=list(si.on_update))
                    changed = True
                new.append(inst)
            if changed:
                blk.instructions = new


# --------------------------------------------------------------------------
# the bass program (input-value independent; built once)
# --------------------------------------------------------------------------
def _build_nc(split=True):
    nc = bass.Bass("TRN2")
    d = {}
    specs = [
        ("x4", [ROWS, L], BF),
        ("xT00", [128, 2, 16, 128], F8), ("xT01", [128, 2, 16, 128], F8),
        ("xT10", [128, 2, 16, 128], F8), ("xT11", [128, 2, 16, 128], F8),
        ("F0", [128, 2, 16, 128], F8), ("F1", [128, 2, 16, 128], F8),
        ("CA", [128, 1425], BF),
        ("G", [64, 2, 8192], F8),
    ]
    specs.append(("Wd0", [128, 4096], F8))
    specs.append(("Wq2", [128, 2048], F8))
    specs.append(("Wq3", [128, 2048], F8))
    for name, shape, dt_ in specs:
        d[name] = nc.dram_tensor(name, shape, dt_, kind="ExternalInput")
    y = nc.dram_tensor("y", [ROWS, L], BF, kind="ExternalOutput")

    with TileContext(nc) as tc:
        from contextlib import ExitStack
        with ExitStack() as ctx:
            const = ctx.enter_context(tc.tile_pool(name="const", bufs=1))
            small = ctx.enter_context(tc.tile_pool(name="small", bufs=1))
            sop = ctx.enter_context(tc.tile_pool(name="sop", bufs=8))

            def cload(eng, name, shape, dt_=FP):
                t = const.tile(shape, dt_, tag=name, name=name)
                eng.dma_start(out=t[:], in_=d[name][:])
                return t

            # ---- ACT queue: act-table preload, packed consts, W half ----
            scr = small.tile([1, 1], FP, tag="scr", name="scr")
            nc.vector.memset(scr[:], 0.0)
            dum = small.tile([1, 1], FP, tag="dum", name="dum")
            nc.scalar.activation(dum[:], scr[:], AF.Silu)
            ca = cload(nc.scalar, "CA", [128, 1425], BF)

            # ---- SP queue: first x4 tiles, F, Wq2, bulk of x4 ----
            x4t = [[const.tile([128, SZ[k]], BF, tag=f"x4_{t}{k}",
                               name=f"x4_{t}{k}") for k in range(NT)]
                   for t in range(2)]

            def x4load(eng, t, k):
                eng.dma_start(
                    out=x4t[t][k][:],
                    in_=d["x4"][t * 128:(t + 1) * 128,
                                OFF[k]:OFF[k] + SZ[k]])

            def x4rhs(t, k, i):
                return x4t[t][k][:, i * 512:(i + 1) * 512]

            Fh = [cload(nc.sync, f"F{h}", [128, 2, 16, 128], F8)
                  for h in range(2)]

            # ---- Pool queue: xT, Wq3, G tiles, tail of x4 ----
            xTg = [[const.tile([128, 2, 16, 128], F8, tag=f"xT{t}{h}",
                               name=f"xT{t}{h}") for h in range(2)]
                   for t in range(2)]
            Gh = [const.tile([64, 2, 4096], F8, tag=f"G{h}", name=f"G{h}")
                  for h in range(2)]
            Wq3 = const.tile([128, 2048], F8, tag="Wq3", name="Wq3")

            def pload(t, name):
                nc.gpsimd.dma_start(out=t[:], in_=d[name][:])

            def gload(h, j):
                # 2D per-j-plane DMAs: one descriptor per partition row
                nc.gpsimd.dma_start(
                    out=Gh[h][:, j, :],
                    in_=d["G"][:, j, h * 4096:(h + 1) * 4096])

            def gload2(h, j):
                nc.sync.dma_start(
                    out=Gh[h][:, j, :],
                    in_=d["G"][:, j, h * 4096:(h + 1) * 4096])

            pload(xTg[0][0], "xT00")
            pload(xTg[0][1], "xT01")
            pload(Wq3, "Wq3")
            gload(0, 1)
            x4load(nc.gpsimd, 0, 1)

            def pool_loads_2():
                pload(xTg[1][0], "xT10")
                gload(1, 0)
                pload(xTg[1][1], "xT11")
                for k in range(3, NT):
                    x4load(nc.gpsimd, 1, k)

            # SP continues: Wd0, first/third x4 tiles, G1 j-plane 1
            Wd0 = cload(nc.sync, "Wd0", [128, 4096], F8)
            x4load(nc.sync, 0, 0)
            x4load(nc.sync, 0, 2)
            gload2(1, 1)
            for k in range(3, NT):
                x4load(nc.sync, 0, k)
            for k in range(0, 3):
                x4load(nc.sync, 1, k)

            # ---- head A: phi -> dtile, h (MLP layer 1) ----
            phiT_sb = small.tile([B_LOC, 128], BF, tag="phiT_sb")
            phiRI = small.tile([64, 8], FP, tag="phiRI")
            h_sb = small.tile([HID, B_LOC], BF, tag="h_sb")
            gbT_sb = small.tile([5, 128], BF, tag="gbT")
            biasvec = small.tile([4, 64], BF, tag="biasvec")
            bt_sb = small.tile([128, 2], FP, tag="bt_sb")
            linwb2 = [small.tile([128, 128], BF, tag=f"lw{t}", name=f"lw{t}")
                      for t in range(2)]
            tmp44 = small.tile([4, 64], BF, tag="tmp44")
            rtmp = [small.tile([64, 64], BF, tag=f"rtmp{i}", name=f"rtmp{i}")
                    for i in range(2)]
            nc.vector.memset(gbT_sb[:], 1.0)  # row 4 stays 1 (1+gamma)
            for t in range(2):
                nc.vector.memset(linwb2[t][:], 0.0)

            pm = tc.alloc_tile_pool(name="ps_mid", bufs=1, space="PSUM")
            ph = tc.alloc_tile_pool(name="ps_head", bufs=2, space="PSUM")
            h_p = ph.tile([HID, B_LOC], FP, tag="hps", name="h_p")
            for kc in range(2):
                nc.tensor.matmul(h_p[:],
                                 lhsT=ca[:, 520 + kc * 64:520 + (kc + 1) * 64],
                                 rhs=ca[:, kc * 4:(kc + 1) * 4],
                                 start=(kc == 0), stop=(kc == 1))
            phiT_p = ph.tile([B_LOC, 128], FP, tag="hps", name="phiT_p")
            for kc in range(2):
                nc.tensor.matmul(phiT_p[:],
                                 lhsT=ca[:, kc * 4:(kc + 1) * 4],
                                 rhs=ca[:, 8 + kc * 128:8 + (kc + 1) * 128],
                                 start=(kc == 0), stop=(kc == 1))
            nc.scalar.activation(h_sb[:], h_p[:], AF.Silu, bias=ca[0:64, 712:713])
            Wq2 = cload(nc.scalar, "Wq2", [128, 2048], F8)
            nc.scalar.dma_start(out=Gh[0][:, 0, :], in_=d["G"][:, 0, 0:4096])
            nc.vector.tensor_copy(phiT_sb[:], phiT_p[:])

            # phiRI[(br,m), 0:4] = re(phi) per batch, [4:8] = im(phi)
            prp = ph.tile([64, 8], BF, tag="hps", name="prp")
            for i in range(2):
                nc.tensor.transpose(prp[0:64, i * 4:(i + 1) * 4],
                                    phiT_sb[:, i * 64:(i + 1) * 64],
                                    ca[0:4, 1293:1297])
            nc.vector.tensor_copy(phiRI[:], prp[:])

            # ---- mid-pipeline state + pools ----
            RT_sb = [small.tile([128, 128], BF, tag=f"RT{t}", name=f"RT{t}")
                     for t in range(2)]
            XS_sb = [[small.tile([128, 64], BF, tag=f"XS{t}{br}",
                                 name=f"XS{t}{br}") for br in range(2)]
                     for t in range(2)]
            spec_sb = [small.tile([64, 256], BF, tag=f"spec{t}",
                                  name=f"spec{t}") for t in range(2)]
            R2f = [small.tile([64, 2, 128], F8, tag=f"R2f{t}", name=f"R2f{t}")
                   for t in range(2)]

            def fwd_mid(t):
                # fwd DFT: 32 DoubleRow matmuls, K=256 per matmul
                rtp = pm.tile([128, 128], FP, tag="mid", name=f"rtp{t}")
                for c in range(32):
                    hh, cc = divmod(c, 16)
                    nc.tensor.matmul(rtp[:],
                                     lhsT=Fh[hh][:, :, cc, :],
                                     rhs=xTg[t][hh][:, :, cc, :],
                                     start=(c == 0), stop=(c == 31),
                                     perf_mode=DR)
                nc.vector.tensor_copy(RT_sb[t][:], rtp[:])

                # XS: fold phi via stacked-diagonal rhs
                for br in range(2):
                    xsp = pm.tile([128, 64], FP, tag="mid", name=f"xsp{t}{br}")
                    psl = slice(br * 64, br * 64 + 64)
                    for j in range(2):
                        for dout in range(2):
                            nc.tensor.matmul(
                                xsp[dout * 64:(dout + 1) * 64, j::2],
                                lhsT=RT_sb[t][psl, j * 64:(j + 1) * 64],
                                rhs=ca[psl, 264 + dout * 32:
                                       264 + (dout + 1) * 32],
                                start=True, stop=True)
                    nc.vector.tensor_copy(XS_sb[t][br][:], xsp[:])

                # spectral: per-(mode, dout) matmuls, N=2, all base-0
                spp = pm.tile([64, 256], FP, tag="mid", name=f"spp{t}")
                for dout in range(2):
                    for br in range(2):
                        for m in range(M):
                            if dout == 0:
                                wsl = Wd0[:, br * 2048 + m * 64:
                                          br * 2048 + (m + 1) * 64]
                            else:
                                wt = Wq2 if br == 0 else Wq3
                                wsl = wt[:, m * 64:(m + 1) * 64]
                            col = dout * 128 + (br * 32 + m) * 2
                            nc.tensor.matmul(
                                spp[0:64, col:col + 2],
                                lhsT=wsl,
                                rhs=XS_sb[t][br][:, m * 2:(m + 1) * 2],
                                start=True, stop=True)
                nc.vector.tensor_copy(spec_sb[t][:], spp[:])

                # R2 transposes -> R2f_dr [64 p=(br,m), 2 j=dout, 128 (j,o)]
                r2p = pm.tile([64, 256], FP, tag="mid", name=f"r2p{t}")
                for dout in range(2):
                    for j in range(2):
                        nc.tensor.matmul(
                            r2p[0:64, dout * 128 + j * 64:
                                dout * 128 + (j + 1) * 64],
                            lhsT=spec_sb[t][0:64,
                                            dout * 128 + j:dout * 128 + 128:2],
                            rhs=ca[0:64, 1361:1425],
                            start=True, stop=True)
                # complex phi rotation (per batch): re' = re*pr - im*pi,
                # im' = re*pi + im*pr, on an SBUF fp16 copy of r2p (avoids
                # the PSUM access penalty on every DVE op)
                from concourse.alu_op_type import AluOpType as AO
                r2s = small.tile([64, 256], BF, tag=f"r2s{t}", name=f"r2s{t}")
                nc.vector.tensor_copy(r2s[:], r2p[:])
                for j in range(2):
                    b = 2 * t + j
                    pr = phiRI[0:64, b:b + 1]
                    pi = phiRI[0:64, 4 + b:5 + b]
                    reb = r2s[0:64, j * 64:(j + 1) * 64]
                    imb = r2s[0:64, 128 + j * 64:128 + (j + 1) * 64]
                    nc.vector.tensor_scalar_mul(rtmp[0][:], imb, pi)
                    nc.vector.scalar_tensor_tensor(
                        R2f[t][:, 0, j * 64:(j + 1) * 64], reb, pr,
                        rtmp[0][:], AO.mult, AO.subtract)
                    nc.vector.tensor_scalar_mul(rtmp[1][:], imb, pr)
                    nc.vector.scalar_tensor_tensor(
                        R2f[t][:, 1, j * 64:(j + 1) * 64], reb, pi,
                        rtmp[1][:], AO.mult, AO.add)

            fwd_mid(0)

            # ---- head B: gbT, bias vector, scaled time weights ----
            gbT_p = ph.tile([4, 128], FP, tag="hps", name="gbT_p")
            nc.tensor.matmul(gbT_p[:], lhsT=h_sb[:], rhs=ca[0:64, 713:841],
                             start=True, stop=True)
            nc.vector.tensor_add(gbT_sb[0:4, :], gbT_p[:], ca[0:4, 841:969])
            rep_p = [ph.tile([128, 64], FP, tag="hps", name=f"rep{t}")
                     for t in range(2)]
            for t in range(2):
                nc.tensor.matmul(rep_p[t][:],
                                 lhsT=ca[0:5, 1033 + t * 128:1033 + (t + 1) * 128],
                                 rhs=gbT_sb[0:5, 0:64], start=True, stop=True)
            # biasvec = gamma*lin_b + lin_b + beta (true scale)
            nc.vector.tensor_mul(tmp44[:], gbT_sb[0:4, 0:64], ca[0:4, 969:1033])
            nc.vector.tensor_add(tmp44[:], tmp44[:], ca[0:4, 969:1033])
            nc.vector.tensor_add(biasvec[:], tmp44[:], gbT_sb[0:4, 64:128])
            # bt_sb[(j,o), t] = biasvec[2t+j, o] via 2 selector matmuls
            btp = ph.tile([128, 2], FP, tag="hps", name="btp")
            for j in range(2):
                nc.tensor.matmul(btp[j * 64:(j + 1) * 64, :],
                                 lhsT=biasvec[:],
                                 rhs=ca[0:4, 1289 + j * 2:1289 + (j + 1) * 2],
                                 start=True, stop=True)
            nc.vector.tensor_copy(bt_sb[:], btp[:])
            # linwb2[t][(j,c),(j,o)] block-diag = lin_w.T*ST*(1+gamma[2t+j])
            # -- multiplies run on gpsimd (emitted into the Pool stream
            # between its DMAs) to keep the DVE queue free for the
            # fwd->XS->spectral->rotation chain that gates the first silu
            for t in range(2):
                for j in range(2):
                    sl = slice(j * 64, (j + 1) * 64)
                    nc.vector.tensor_mul(linwb2[t][sl, sl], ca[sl, 648:712],
                                         rep_p[t][sl, :])
            pool_loads_2()
            ph.release()

            poA = tc.alloc_tile_pool(name="ps_oa", bufs=1, space="PSUM")
            poB = tc.alloc_tile_pool(name="ps_ob", bufs=1, space="PSUM")

            def out_tile(t, k):
                po = poA if k % 2 == 0 else poB
                sz = SZ[k]
                nch = sz // 512
                pos = po.tile([128, sz], FP, tag="po", name=f"po{t}{k}")
                for i in range(nch):
                    nc.tensor.matmul(
                        pos[:, i * 512:(i + 1) * 512],
                        lhsT=linwb2[t][:],
                        rhs=x4rhs(t, k, i),
                        start=True, stop=False)
                gh = 0 if k < 3 else 1
                gof = OFF[k] - gh * 4096
                for i in range(nch):
                    nc.tensor.matmul(
                        pos[:, i * 512:(i + 1) * 512],
                        lhsT=R2f[t][:],
                        rhs=Gh[gh][:, :, gof + i * 512:gof + (i + 1) * 512],
                        start=False, stop=True, perf_mode=DR)
                so = sop.tile([128, sz], BF, tag="so")
                nc.scalar.activation(so[:], pos[:], AF.Silu,
                                     bias=bt_sb[:, t:t + 1], scale=DESCALE)
                if (t, k) == (1, 4):
                    # split the late big store across both queues
                    for hf, eng in ((0, nc.gpsimd), (1, nc.sync)):
                        nc.gpsimd if hf else nc.sync
                        eng.dma_start(
                            out=y[t * 128:(t + 1) * 128,
                                  OFF[k] + hf * 1024:OFF[k] + (hf + 1) * 1024],
                            in_=so[:, hf * 1024:(hf + 1) * 1024])
                else:
                    eng = nc.gpsimd if (t * NT + k) % 2 == 0 else nc.sync
                    eng.dma_start(
                        out=y[t * 128:(t + 1) * 128, OFF[k]:OFF[k] + sz],
                        in_=so[:])

            for k in range(4):
                out_tile(0, k)
            fwd_mid(1)
            for k in range(4, NT):
                out_tile(0, k)
            for k in range(NT):
                out_tile(1, k)
            poB.release()
            poA.release()
            pm.release()

    if split:
        _split_multiwait(nc)
    return nc


_NC = None


def _get_nc():
    global _NC
    if _NC is None:
        _NC = _build_nc()
    return _NC


def _core_inputs(x, emb, consts, core):
    b0 = core * B_LOC
    m = dict(consts)
    m.update(_stage_x(np.ascontiguousarray(x[b0:b0 + B_LOC])))
    eT = emb[b0:b0 + B_LOC].T.astype(np.float32)
    CA = consts["CA"].copy()
    CA[:, 0:8] = eT.reshape(2, 128, B_LOC).transpose(1, 0, 2).reshape(
        128, 8).astype(np.float16)
    m["CA"] = CA
    return m


def kernel(**inputs):
    inputs = {k: np.asarray(v) for k, v in inputs.items()}
    x, emb = inputs["x"], inputs["emb"]
    consts = _build_constants(**{k: v for k, v in inputs.items()
                                 if k not in ("x", "emb")})
    nc = _get_nc()

    in_maps = [_core_inputs(x, emb, consts, core) for core in range(N_CORES)]
    res = run_bass_kernel_spmd(nc, in_maps, core_ids=list(range(N_CORES)))
    out = np.empty((B, C, L), np.float32)
    for core in range(N_CORES):
        b0 = core * B_LOC
        out[b0:b0 + B_LOC] = res.results[core]["y"].astype(
            np.float32).reshape(B_LOC, C, L)
    return out
